# revision 12
# baseline (speedup 1.0000x reference)
"""Multi-head causal attention on 8 TRN2 NeuronCores.

Problem: B=4, T=2048, D=1024, H=16 heads of 64. Sharding: core c handles
batch c//2 and head-group c%2 (8 heads = 512 cols of the concat dim). Each
core computes its partial output projection o_g @ Wo_g^T; the host sums the
two partials per batch and adds the bias.

Per-core kernel (bf16 matmuls, fp32 accumulation):
  QT[hd, t], KT[hd, t]; V[t, hd] with a ones column per head (V_aug).
  scoresT[s, q] = K Q^T per stripe of 128 keys (causal lower block-row),
  exp fused into the PSUM->SBUF activation on ScalarE giving PT[s, q]
  stored in fp8e4 (ones-column denominator uses the same quantized values,
  so numerator/denominator errors track); causal mask = multiply the
  diagonal 128-block by an upper-triangular 0/1 mask after exp.
  PV: out[q, 0:65|65:130] = P @ V_aug accumulated over s-tiles; col 64 of
  each half is the softmax denominator. Normalize with a per-partition
  reciprocal, DMA-transpose [128,128] into oT[hd, t].
  proj: partial[t, :] = oT^T @ WoT_g.

Schedule: fp8 PT halves its SBUF footprint, so FOUR pt buffers fit and
each pair's PV can be deferred into the next pair's region. That balances
the PE against the ~42us/pair ScalarE exp stream:
  A (pair 0 scores): m0 chains k-major-pipelined against the input DMA,
     v tiles 0-11, m1 chains.
  B (pair 1 scores): v tiles 12-15, pv0, m2 chains.
  C (pair 2 scores): m3 chains, pv1, pv2.
  D (pair 3 scores): pv3 + output projection lagged two stripes.
A dummy exp at t=0 hoists the ~2.7us ACT table load; oT transposes
round-robin over three DMA queues to avoid descriptor-flood stalls.
"""

import numpy as np
import ml_dtypes
from contextlib import ExitStack

import concourse.mybir as mybir
import concourse.tile as tile
from concourse import bacc
from concourse.bass_utils import run_bass_kernel_spmd

F32 = mybir.dt.float32
BF16 = mybir.dt.bfloat16
F8 = mybir.dt.float8e4

FP8_PT = True  # store exp'd scores in fp8e4 (enables pv deferral)

B, T, D, H = 4, 2048, 1024, 16
HD = 64          # head dim
HG = 8           # heads per core
GW = HG * HD     # 512, group width
NT = T // 128    # 16 t-tiles
NK = D // 128    # 8 d-tiles
N_CORES = 8

PT_DT = F8 if FP8_PT else BF16

# ragged PT stripe offsets: stripe j holds cols q=128j..2048
_PT_OFF = [2048 * j - 64 * j * (j - 1) for j in range(NT + 1)]
PT_LEN = _PT_OFF[NT]  # 17408


def _build():
    nc = bacc.Bacc("TRN2", target_bir_lowering=False, debug=False,
                   num_devices=N_CORES)
    xT_d = nc.dram_tensor("xT", [D, T], BF16, kind="ExternalInput").ap()
    wq_d = nc.dram_tensor("wq", [D, GW], BF16, kind="ExternalInput").ap()
    wk_d = nc.dram_tensor("wk", [D, GW], BF16, kind="ExternalInput").ap()
    wv_d = nc.dram_tensor("wv", [D, GW], BF16, kind="ExternalInput").ap()
    wo_d = nc.dram_tensor("woT", [GW, D], BF16, kind="ExternalInput").ap()
    tri_d = nc.dram_tensor("tri", [128, 128], PT_DT, kind="ExternalInput").ap()
    out_d = nc.dram_tensor("out", [T, D], BF16, kind="ExternalOutput").ap()

    with tile.TileContext(nc) as tc, ExitStack() as ctx:
        perm = ctx.enter_context(tc.tile_pool(name="perm", bufs=1))
        psA = ctx.enter_context(tc.tile_pool(name="psA", bufs=2, space="PSUM"))
        psB = ctx.enter_context(tc.tile_pool(name="psB", bufs=2, space="PSUM"))
        ps_o = ctx.enter_context(tc.tile_pool(name="ps_o", bufs=2, space="PSUM"))

        tri = perm.tile([128, 128], PT_DT, tag="tri")

        qT = perm.tile([128, 4, T], BF16, tag="qT")
        kT = perm.tile([128, 4, T], BF16, tag="kT")
        vsb = perm.tile([128, NT, HG * (HD + 1)], BF16, tag="vsb")
        wob = perm.tile([128, 4, D], BF16, tag="wob")
        oT = perm.tile([128, 4, T], BF16, tag="oT")

        # dummy exp at t=0 pulls the ACT table load off the critical path
        dummy = perm.tile([128, 8], F32, tag="dummy")
        nc.vector.memset(dummy[:, 0:4], 0.0)
        nc.scalar.activation(dummy[:, 4:8], dummy[:, 0:4],
                             mybir.ActivationFunctionType.Exp, scale=1.0)

        # ones columns for V_aug
        vcols = vsb.rearrange("p j (h c) -> p j h c", c=HD + 1)
        nc.vector.memset(vcols[:, :, :, HD:HD + 1], 1.0)

        def scores_chunk(pair, pts, j, qa, w):
            """one <=1024-wide chunk of scoresT for both heads of a stripe
            (the heads' K=64 matmuls sit on disjoint PE row groups)"""
            m = pair
            pss = [psA.tile([128, 1024], F32, tag="psA",
                            name=f"s{pair}_{hh}_{j}_{qa}") for hh in range(2)]
            a = 0
            while a < w:
                b = min(a + 512, w)
                for hh in range(2):
                    base = 64 * hh
                    nc.tensor.matmul(
                        pss[hh][:, a:b],
                        kT[base:base + 64, m, 128 * j:128 * (j + 1)],
                        qT[base:base + 64, m, qa + a:qa + b],
                        start=True, stop=True,
                        tile_position=(base, 0))
                a = b
            o0 = _PT_OFF[j] + (qa - 128 * j)
            for hh in range(2):
                nc.scalar.activation(
                    pts[hh][:, o0:o0 + w], pss[hh][:, :w],
                    mybir.ActivationFunctionType.Exp, scale=0.125)
            if qa == 128 * j:
                # first chunk holds the diagonal block: causal mask
                for hh in range(2):
                    nc.vector.tensor_mul(pts[hh][:, o0:o0 + 128],
                                         pts[hh][:, o0:o0 + 128], tri[:])

        def stripe_chunks(j):
            out = []
            qa = 128 * j
            first = True
            while qa < T:
                w = min(1024 - (qa % 1024) if first else 1024, T - qa)
                first = False
                out.append((qa, w))
                qa += w
            return out

        def scores_stripe(pair, pts, j):
            for qa, w in stripe_chunks(j):
                scores_chunk(pair, pts, j, qa, w)

        tq_rr = [0]

        def pv_i(pair, pts, i, smp):
            """PV + normalize + DMA-transpose into oT for one q-tile"""
            po = ps_o.tile([128, 2 * (HD + 1)], F32, tag="po")
            for hh in range(2):
                h = 2 * pair + hh
                pt = pts[hh]
                c0 = (HD + 1) * hh
                for j in range(i + 1):
                    nc.tensor.matmul(
                        po[:, c0:c0 + HD + 1],
                        pt[:, _PT_OFF[j] + 128 * (i - j):
                           _PT_OFF[j] + 128 * (i - j) + 128],
                        vsb[:, j, (HD + 1) * h:(HD + 1) * (h + 1)],
                        start=(j == 0), stop=(j == i))
            recip = smp.tile([128, 2], F32, tag="recip")
            pov = po.rearrange("p (h c) -> p h c", c=HD + 1)
            nc.vector.reciprocal(recip[:], pov[:, :, HD])
            onat = smp.tile([128, 128], BF16, tag="onat")
            for hh in range(2):
                c0 = (HD + 1) * hh
                nc.vector.tensor_scalar_mul(
                    onat[:, 64 * hh:64 * hh + 64],
                    po[:, c0:c0 + HD], recip[:, hh:hh + 1])
            nc.sync.dma_start(oT[:, pair, 128 * i:128 * (i + 1)],
                              onat[:], transpose=True)

        with tc.tile_pool(name="ph1", bufs=1) as ph1, \
             tc.tile_pool(name="ptp", bufs=(4 if FP8_PT else 2)) as ptp, \
             tc.tile_pool(name="sm", bufs=16) as smp, \
             tc.tile_pool(name="outp", bufs=2) as outp:
            xT = ph1.tile([128, NK, T], BF16, tag="xT")
            wqb = ph1.tile([128, NK, GW], BF16, tag="wqb")
            wkb = ph1.tile([128, NK, GW], BF16, tag="wkb")
            wvb = ph1.tile([128, NK, GW], BF16, tag="wvb")

            # spread the input loads over all three queues, k-major so the
            # m0 chains can pipeline against the arriving k-tiles
            qs = [nc.sync, nc.scalar, nc.gpsimd]
            for k in range(NK):
                qs[k % 3].dma_start(xT[:, k, :],
                                    xT_d[128 * k:128 * (k + 1), :])
                qs[(k + 1) % 3].dma_start(wqb[:, k, :],
                                          wq_d[128 * k:128 * (k + 1), :])
                qs[(k + 2) % 3].dma_start(wkb[:, k, :],
                                          wk_d[128 * k:128 * (k + 1), :])
            qs[1].dma_start(tri[:], tri_d[:])
            for k in range(NK):
                qs[k % 3].dma_start(wvb[:, k, :],
                                    wv_d[128 * k:128 * (k + 1), :])
            for k in range(4):
                qs[(k + 1) % 3].dma_start(wob[:, k, :],
                                          wo_d[128 * k:128 * (k + 1), :])

            def qk_chain(mt, c, which):
                """one 8-matmul contraction chain: q (which=0) or k proj,
                m-tile mt, column chunk c (512 wide)"""
                wbt, dst = ((wqb, qT), (wkb, kT))[which]
                ps = psB.tile([128, 512], F32, tag="psB")
                for k in range(NK):
                    nc.tensor.matmul(
                        ps[:], wbt[:, k, 128 * mt:128 * (mt + 1)],
                        xT[:, k, 512 * c:512 * (c + 1)],
                        start=(k == 0), stop=(k == NK - 1))
                nc.vector.tensor_copy(dst[:, mt, 512 * c:512 * (c + 1)], ps[:])

            def v_jtile(j):
                ps = psB.tile([128, 512], F32, tag="psB")
                for k in range(NK):
                    nc.tensor.matmul(ps[:],
                                     xT[:, k, 128 * j:128 * (j + 1)],
                                     wvb[:, k, :],
                                     start=(k == 0), stop=(k == NK - 1))
                nc.vector.tensor_copy(vcols[:, j, :, :HD], ps[:])

            def proj_i(i):
                ost = outp.tile([128, D], BF16, tag="ost", name=f"ost{i}")
                for n in range(2):
                    ps = psB.tile([128, 512], F32, tag="psB")
                    for k in range(4):
                        nc.tensor.matmul(ps[:],
                                         oT[:, k, 128 * i:128 * (i + 1)],
                                         wob[:, k, 512 * n:512 * (n + 1)],
                                         start=(k == 0), stop=(k == 3))
                    nc.vector.tensor_copy(ost[:, 512 * n:512 * (n + 1)],
                                          ps[:])
                nc.scalar.dma_start(out_d[128 * i:128 * (i + 1), :], ost[:])

            def pt_alloc(pair):
                return [ptp.tile([128, PT_LEN], PT_DT, tag="pt",
                                 name=f"pt{pair}_{hh}") for hh in range(2)]

            # ---- region A: pair 0 scores ----
            pts0 = pt_alloc(0)
            # m0 chains for q/k cols 0:1024, k-major so they pipeline
            # against the arriving xT/wq/wk k-tiles
            psq01 = psA.tile([128, 1024], F32, tag="psA", name="pro_q01")
            psk01 = psA.tile([128, 1024], F32, tag="psA", name="pro_k01")
            for k in range(NK):
                st, sp = (k == 0), (k == NK - 1)
                for c in range(2):
                    nc.tensor.matmul(psq01[:, 512 * c:512 * (c + 1)],
                                     wqb[:, k, 0:128],
                                     xT[:, k, 512 * c:512 * (c + 1)],
                                     start=st, stop=sp)
                    nc.tensor.matmul(psk01[:, 512 * c:512 * (c + 1)],
                                     wkb[:, k, 0:128],
                                     xT[:, k, 512 * c:512 * (c + 1)],
                                     start=st, stop=sp)
            nc.vector.tensor_copy(qT[:, 0, 0:1024], psq01[:])
            nc.vector.tensor_copy(kT[:, 0, 0:1024], psk01[:])
            scores_chunk(0, pts0, 0, 0, 1024)
            for c in range(2, 4):
                qk_chain(0, c, 0)
                qk_chain(0, c, 1)
            scores_chunk(0, pts0, 0, 1024, 1024)
            v_jtile(0)
            if not FP8_PT:
                pv_i(0, pts0, 0, smp)
            # stripes in bursts of two: fewer 64<->128 PE-mode switches
            for j0 in range(1, NT, 2):
                js = [j0] if j0 + 1 >= NT else [j0, j0 + 1]
                for j in js:
                    scores_stripe(0, pts0, j)
                for j in js:
                    if j < 12:
                        v_jtile(j)
                    if j >= 8:
                        qk_chain(1, (j - 8) // 2, (j - 8) % 2)
                if not FP8_PT:
                    for j in js:
                        pv_i(0, pts0, j, smp)

            # ---- region B: pair 1 scores (+ pv0 when deferred) ----
            pts1 = pt_alloc(1)
            for j0 in range(0, NT, 2):
                js = [j0, j0 + 1]
                for j in js:
                    scores_stripe(1, pts1, j)
                for j in js:
                    if j < 4:
                        v_jtile(12 + j)
                    if j >= 8:
                        qk_chain(2, (j - 8) // 2, (j - 8) % 2)
                for j in js:
                    pv_i(0 if FP8_PT else 1, pts0 if FP8_PT else pts1,
                         j, smp)

            # ---- region C: pair 2 scores + pv1/pv2 ----
            pts2 = pt_alloc(2)
            for j0 in range(0, NT, 2):
                js = [j0, j0 + 1]
                for j in js:
                    scores_stripe(2, pts2, j)
                for j in js:
                    if j < 8:
                        qk_chain(3, j // 2, j % 2)
                if FP8_PT:
                    for j in js:
                        pv_i(1, pts1, j, smp)
                for j in js:
                    pv_i(2, pts2, j, smp)

            # ---- region D: pair 3 scores + pv3 + lagged out proj ----
            pts3 = pt_alloc(3)
            for j0 in range(0, NT, 2):
                js = [j0, j0 + 1]
                for j in js:
                    scores_stripe(3, pts3, j)
                for j in js:
                    pv_i(3, pts3, j, smp)
                for j in js:
                    if j >= 2:
                        proj_i(j - 2)
            proj_i(NT - 2)
            proj_i(NT - 1)

    nc.compile()
    return nc


_NC_CACHE = None


def _get_nc():
    global _NC_CACHE
    if _NC_CACHE is None:
        _NC_CACHE = _build()
    return _NC_CACHE


def _prep_in_maps(x, Wq, Wk, Wv, Wo):
    bf = ml_dtypes.bfloat16
    tri_dt = ml_dtypes.float8_e4m3 if FP8_PT else bf
    tri = np.triu(np.ones((128, 128), dtype=tri_dt))
    in_maps = []
    for c in range(N_CORES):
        b, g = c // 2, c % 2
        hsl = slice(HG * g, HG * (g + 1))
        in_maps.append({
            "xT": np.ascontiguousarray(x[b].T).astype(bf),
            "wq": np.ascontiguousarray(
                Wq[hsl].transpose(1, 0, 2).reshape(D, GW)).astype(bf),
            "wk": np.ascontiguousarray(
                Wk[hsl].transpose(1, 0, 2).reshape(D, GW)).astype(bf),
            "wv": np.ascontiguousarray(
                Wv[hsl].transpose(1, 0, 2).reshape(D, GW)).astype(bf),
            "woT": np.ascontiguousarray(
                Wo[:, GW * g:GW * (g + 1)].T).astype(bf),
            "tri": tri,
        })
    return in_maps


def kernel(x, Wq, Wk, Wv, Wo, bo, _trace=False, _tmpdir=None):
    nc = _get_nc()
    x = np.asarray(x, dtype=np.float32)
    bo = np.asarray(bo, dtype=np.float32)
    in_maps = _prep_in_maps(x, np.asarray(Wq, np.float32),
                            np.asarray(Wk, np.float32),
                            np.asarray(Wv, np.float32),
                            np.asarray(Wo, np.float32))
    res = run_bass_kernel_spmd(nc, in_maps, core_ids=list(range(N_CORES)),
                               trace=_trace, tmpdir=_tmpdir)
    out = np.empty((B, T, D), dtype=np.float32)
    for b in range(B):
        out[b] = (res.results[2 * b]["out"].astype(np.float32)
                  + res.results[2 * b + 1]["out"].astype(np.float32) + bo)
    if _trace:
        return out, res
    return out


# revision 13
# speedup vs baseline: 1.0176x; 1.0176x over previous
"""Multi-head causal attention on 8 TRN2 NeuronCores.

Problem: B=4, T=2048, D=1024, H=16 heads of 64. Sharding: core c handles
batch c//2 and head-group c%2 (8 heads = 512 cols of the concat dim). Each
core computes its partial output projection o_g @ Wo_g^T; the host sums the
two partials per batch and adds the bias.

Per-core kernel (bf16 matmuls, fp32 accumulation):
  QT[hd, t], KT[hd, t]; V[t, hd] with a ones column per head (V_aug).
  scoresT[s, q] = K Q^T per stripe of 128 keys (causal lower block-row),
  exp fused into the PSUM->SBUF activation on ScalarE giving PT[s, q]
  stored in fp8e4 (ones-column denominator uses the same quantized values,
  so numerator/denominator errors track); causal mask = multiply the
  diagonal 128-block by an upper-triangular 0/1 mask after exp.
  PV: out[q, 0:65|65:130] = P @ V_aug accumulated over s-tiles; col 64 of
  each half is the softmax denominator. Normalize with a per-partition
  reciprocal, DMA-transpose [128,128] into oT[hd, t].
  proj: partial[t, :] = oT^T @ WoT_g.

Schedule: fp8 PT halves its SBUF footprint, so FOUR pt buffers fit and
each pair's PV can be deferred into the next pair's region. That balances
the PE against the ~42us/pair ScalarE exp stream:
  A (pair 0 scores): m0 chains k-major-pipelined against the input DMA,
     v tiles 0-11, m1 chains.
  B (pair 1 scores): v tiles 12-15, pv0, m2 chains.
  C (pair 2 scores): m3 chains, pv1, pv2.
  D (pair 3 scores): pv3 + output projection lagged two stripes.
A dummy exp at t=0 hoists the ~2.7us ACT table load; oT transposes
round-robin over three DMA queues to avoid descriptor-flood stalls.
"""

import numpy as np
import ml_dtypes
from contextlib import ExitStack

import concourse.mybir as mybir
import concourse.tile as tile
from concourse import bacc
from concourse.bass_utils import run_bass_kernel_spmd

F32 = mybir.dt.float32
BF16 = mybir.dt.bfloat16
F8 = mybir.dt.float8e4

FP8_PT = True  # store exp'd scores in fp8e4 (enables pv deferral)

B, T, D, H = 4, 2048, 1024, 16
HD = 64          # head dim
HG = 8           # heads per core
GW = HG * HD     # 512, group width
NT = T // 128    # 16 t-tiles
NK = D // 128    # 8 d-tiles
N_CORES = 8

PT_DT = F8 if FP8_PT else BF16

# ragged PT stripe offsets: stripe j holds cols q=128j..2048
_PT_OFF = [2048 * j - 64 * j * (j - 1) for j in range(NT + 1)]
PT_LEN = _PT_OFF[NT]  # 17408


def _build():
    nc = bacc.Bacc("TRN2", target_bir_lowering=False, debug=False,
                   num_devices=N_CORES)
    xT_d = nc.dram_tensor("xT", [D, T], BF16, kind="ExternalInput").ap()
    wq_d = nc.dram_tensor("wq", [D, GW], BF16, kind="ExternalInput").ap()
    wk_d = nc.dram_tensor("wk", [D, GW], BF16, kind="ExternalInput").ap()
    wv_d = nc.dram_tensor("wv", [D, GW], BF16, kind="ExternalInput").ap()
    wo_d = nc.dram_tensor("woT", [GW, D], BF16, kind="ExternalInput").ap()
    tri_d = nc.dram_tensor("tri", [128, 128], PT_DT, kind="ExternalInput").ap()
    out_d = nc.dram_tensor("out", [T, D], BF16, kind="ExternalOutput").ap()

    with tile.TileContext(nc) as tc, ExitStack() as ctx:
        perm = ctx.enter_context(tc.tile_pool(name="perm", bufs=1))
        psA = ctx.enter_context(tc.tile_pool(name="psA", bufs=2, space="PSUM"))
        psB = ctx.enter_context(tc.tile_pool(name="psB", bufs=2, space="PSUM"))
        ps_o = ctx.enter_context(tc.tile_pool(name="ps_o", bufs=2, space="PSUM"))

        tri = perm.tile([128, 128], PT_DT, tag="tri")

        qT = perm.tile([128, 4, T], BF16, tag="qT")
        kT = perm.tile([128, 4, T], BF16, tag="kT")
        vsb = perm.tile([128, NT, HG * (HD + 1)], BF16, tag="vsb")
        wob = perm.tile([128, 4, D], BF16, tag="wob")
        oT = perm.tile([128, 4, T], BF16, tag="oT")

        # dummy exp at t=0 pulls the ACT table load off the critical path
        dummy = perm.tile([128, 8], F32, tag="dummy")
        nc.vector.memset(dummy[:, 0:4], 0.0)
        nc.scalar.activation(dummy[:, 4:8], dummy[:, 0:4],
                             mybir.ActivationFunctionType.Exp, scale=1.0)

        # ones columns for V_aug
        vcols = vsb.rearrange("p j (h c) -> p j h c", c=HD + 1)
        nc.vector.memset(vcols[:, :, :, HD:HD + 1], 1.0)

        def scores_chunk(pair, pts, j, qa, w):
            """one <=1024-wide chunk of scoresT for both heads of a stripe
            (the heads' K=64 matmuls sit on disjoint PE row groups)"""
            m = pair
            pss = [psA.tile([128, 1024], F32, tag="psA",
                            name=f"s{pair}_{hh}_{j}_{qa}") for hh in range(2)]
            a = 0
            while a < w:
                b = min(a + 512, w)
                for hh in range(2):
                    base = 64 * hh
                    nc.tensor.matmul(
                        pss[hh][:, a:b],
                        kT[base:base + 64, m, 128 * j:128 * (j + 1)],
                        qT[base:base + 64, m, qa + a:qa + b],
                        start=True, stop=True,
                        tile_position=(base, 0))
                a = b
            o0 = _PT_OFF[j] + (qa - 128 * j)
            for hh in range(2):
                nc.scalar.activation(
                    pts[hh][:, o0:o0 + w], pss[hh][:, :w],
                    mybir.ActivationFunctionType.Exp, scale=0.125)
            if qa == 128 * j:
                # first chunk holds the diagonal block: causal mask
                for hh in range(2):
                    nc.vector.tensor_mul(pts[hh][:, o0:o0 + 128],
                                         pts[hh][:, o0:o0 + 128], tri[:])

        def stripe_chunks(j):
            out = []
            qa = 128 * j
            first = True
            while qa < T:
                w = min(1024 - (qa % 1024) if first else 1024, T - qa)
                first = False
                out.append((qa, w))
                qa += w
            return out

        def scores_stripe(pair, pts, j):
            for qa, w in stripe_chunks(j):
                scores_chunk(pair, pts, j, qa, w)

        tq_rr = [0]

        def pv_i(pair, pts, i, smp):
            """PV + normalize + DMA-transpose into oT for one q-tile"""
            po = ps_o.tile([128, 2 * (HD + 1)], F32, tag="po")
            for hh in range(2):
                h = 2 * pair + hh
                pt = pts[hh]
                c0 = (HD + 1) * hh
                for j in range(i + 1):
                    nc.tensor.matmul(
                        po[:, c0:c0 + HD + 1],
                        pt[:, _PT_OFF[j] + 128 * (i - j):
                           _PT_OFF[j] + 128 * (i - j) + 128],
                        vsb[:, j, (HD + 1) * h:(HD + 1) * (h + 1)],
                        start=(j == 0), stop=(j == i))
            recip = smp.tile([128, 2], F32, tag="recip")
            pov = po.rearrange("p (h c) -> p h c", c=HD + 1)
            nc.vector.reciprocal(recip[:], pov[:, :, HD])
            onat = smp.tile([128, 128], BF16, tag="onat")
            for hh in range(2):
                c0 = (HD + 1) * hh
                nc.vector.tensor_scalar_mul(
                    onat[:, 64 * hh:64 * hh + 64],
                    po[:, c0:c0 + HD], recip[:, hh:hh + 1])
            nc.sync.dma_start(oT[:, pair, 128 * i:128 * (i + 1)],
                              onat[:], transpose=True)

        with tc.tile_pool(name="ph1", bufs=1) as ph1, \
             tc.tile_pool(name="ptp", bufs=(4 if FP8_PT else 2)) as ptp, \
             tc.tile_pool(name="sm", bufs=16) as smp, \
             tc.tile_pool(name="outp", bufs=2) as outp:
            xT = ph1.tile([128, NK, T], BF16, tag="xT")
            wqb = ph1.tile([128, NK, GW], BF16, tag="wqb")
            wkb = ph1.tile([128, NK, GW], BF16, tag="wkb")
            wvb = ph1.tile([128, NK, GW], BF16, tag="wvb")

            # k-major interleaved loads so the m0 chains pipeline with DMA
            qs = [nc.sync, nc.scalar, nc.gpsimd]
            for k in range(NK):
                qs[0].dma_start(xT[:, k, :], xT_d[128 * k:128 * (k + 1), :])
                qs[1].dma_start(wqb[:, k, :], wq_d[128 * k:128 * (k + 1), :])
                qs[2].dma_start(wkb[:, k, :], wk_d[128 * k:128 * (k + 1), :])
            qs[1].dma_start(tri[:], tri_d[:])
            for k in range(NK):
                qs[(k % 2) + 1].dma_start(wvb[:, k, :],
                                          wv_d[128 * k:128 * (k + 1), :])
            for k in range(4):
                qs[0].dma_start(wob[:, k, :], wo_d[128 * k:128 * (k + 1), :])

            def qk_chain(mt, c, which):
                """one 8-matmul contraction chain: q (which=0) or k proj,
                m-tile mt, column chunk c (512 wide)"""
                wbt, dst = ((wqb, qT), (wkb, kT))[which]
                ps = psB.tile([128, 512], F32, tag="psB")
                for k in range(NK):
                    nc.tensor.matmul(
                        ps[:], wbt[:, k, 128 * mt:128 * (mt + 1)],
                        xT[:, k, 512 * c:512 * (c + 1)],
                        start=(k == 0), stop=(k == NK - 1))
                nc.vector.tensor_copy(dst[:, mt, 512 * c:512 * (c + 1)], ps[:])

            def v_jtile(j):
                ps = psB.tile([128, 512], F32, tag="psB")
                for k in range(NK):
                    nc.tensor.matmul(ps[:],
                                     xT[:, k, 128 * j:128 * (j + 1)],
                                     wvb[:, k, :],
                                     start=(k == 0), stop=(k == NK - 1))
                nc.vector.tensor_copy(vcols[:, j, :, :HD], ps[:])

            def proj_i(i):
                ost = outp.tile([128, D], BF16, tag="ost", name=f"ost{i}")
                for n in range(2):
                    ps = psB.tile([128, 512], F32, tag="psB")
                    for k in range(4):
                        nc.tensor.matmul(ps[:],
                                         oT[:, k, 128 * i:128 * (i + 1)],
                                         wob[:, k, 512 * n:512 * (n + 1)],
                                         start=(k == 0), stop=(k == 3))
                    nc.vector.tensor_copy(ost[:, 512 * n:512 * (n + 1)],
                                          ps[:])
                nc.scalar.dma_start(out_d[128 * i:128 * (i + 1), :], ost[:])

            def pt_alloc(pair):
                return [ptp.tile([128, PT_LEN], PT_DT, tag="pt",
                                 name=f"pt{pair}_{hh}") for hh in range(2)]

            # ---- region A: pair 0 scores ----
            pts0 = pt_alloc(0)
            # m0 chains for q/k cols 0:1024, k-major so they pipeline
            # against the arriving xT/wq/wk k-tiles
            psq01 = psA.tile([128, 1024], F32, tag="psA", name="pro_q01")
            psk01 = psA.tile([128, 1024], F32, tag="psA", name="pro_k01")
            for k in range(NK):
                st, sp = (k == 0), (k == NK - 1)
                for c in range(2):
                    nc.tensor.matmul(psq01[:, 512 * c:512 * (c + 1)],
                                     wqb[:, k, 0:128],
                                     xT[:, k, 512 * c:512 * (c + 1)],
                                     start=st, stop=sp)
                    nc.tensor.matmul(psk01[:, 512 * c:512 * (c + 1)],
                                     wkb[:, k, 0:128],
                                     xT[:, k, 512 * c:512 * (c + 1)],
                                     start=st, stop=sp)
            nc.vector.tensor_copy(qT[:, 0, 0:1024], psq01[:])
            nc.vector.tensor_copy(kT[:, 0, 0:1024], psk01[:])
            scores_chunk(0, pts0, 0, 0, 1024)
            for c in range(2, 4):
                qk_chain(0, c, 0)
                qk_chain(0, c, 1)
            scores_chunk(0, pts0, 0, 1024, 1024)
            v_jtile(0)
            if not FP8_PT:
                pv_i(0, pts0, 0, smp)
            for j in range(1, NT):
                scores_stripe(0, pts0, j)
                if j < 12:
                    v_jtile(j)
                if j >= 8:
                    qk_chain(1, (j - 8) // 2, (j - 8) % 2)
                if not FP8_PT:
                    pv_i(0, pts0, j, smp)

            # ---- region B: pair 1 scores (+ pv0 when deferred) ----
            pts1 = pt_alloc(1)
            for j in range(NT):
                scores_stripe(1, pts1, j)
                if j < 4:
                    v_jtile(12 + j)
                if j >= 8:
                    qk_chain(2, (j - 8) // 2, (j - 8) % 2)
                pv_i(0 if FP8_PT else 1, pts0 if FP8_PT else pts1, j, smp)

            # ---- region C: pair 2 scores + pv1/pv2 ----
            pts2 = pt_alloc(2)
            for j in range(NT):
                scores_stripe(2, pts2, j)
                if j < 8:
                    qk_chain(3, j // 2, j % 2)
                if FP8_PT:
                    pv_i(1, pts1, j, smp)
                pv_i(2, pts2, j, smp)

            # ---- region D: pair 3 scores + pv3 + lagged out proj ----
            pts3 = pt_alloc(3)
            for j in range(NT):
                scores_stripe(3, pts3, j)
                pv_i(3, pts3, j, smp)
                if j >= 2:
                    proj_i(j - 2)
            proj_i(NT - 2)
            proj_i(NT - 1)

    nc.compile()
    return nc


_NC_CACHE = None


def _get_nc():
    global _NC_CACHE
    if _NC_CACHE is None:
        _NC_CACHE = _build()
    return _NC_CACHE


def _prep_in_maps(x, Wq, Wk, Wv, Wo):
    bf = ml_dtypes.bfloat16
    tri_dt = ml_dtypes.float8_e4m3 if FP8_PT else bf
    tri = np.triu(np.ones((128, 128), dtype=tri_dt))
    in_maps = []
    for c in range(N_CORES):
        b, g = c // 2, c % 2
        hsl = slice(HG * g, HG * (g + 1))
        in_maps.append({
            "xT": np.ascontiguousarray(x[b].T).astype(bf),
            "wq": np.ascontiguousarray(
                Wq[hsl].transpose(1, 0, 2).reshape(D, GW)).astype(bf),
            "wk": np.ascontiguousarray(
                Wk[hsl].transpose(1, 0, 2).reshape(D, GW)).astype(bf),
            "wv": np.ascontiguousarray(
                Wv[hsl].transpose(1, 0, 2).reshape(D, GW)).astype(bf),
            "woT": np.ascontiguousarray(
                Wo[:, GW * g:GW * (g + 1)].T).astype(bf),
            "tri": tri,
        })
    return in_maps


def kernel(x, Wq, Wk, Wv, Wo, bo, _trace=False, _tmpdir=None):
    nc = _get_nc()
    x = np.asarray(x, dtype=np.float32)
    bo = np.asarray(bo, dtype=np.float32)
    in_maps = _prep_in_maps(x, np.asarray(Wq, np.float32),
                            np.asarray(Wk, np.float32),
                            np.asarray(Wv, np.float32),
                            np.asarray(Wo, np.float32))
    res = run_bass_kernel_spmd(nc, in_maps, core_ids=list(range(N_CORES)),
                               trace=_trace, tmpdir=_tmpdir)
    out = np.empty((B, T, D), dtype=np.float32)
    for b in range(B):
        out[b] = (res.results[2 * b]["out"].astype(np.float32)
                  + res.results[2 * b + 1]["out"].astype(np.float32) + bo)
    if _trace:
        return out, res
    return out


# revision 14
# speedup vs baseline: 1.0410x; 1.0230x over previous
"""Multi-head causal attention on 8 TRN2 NeuronCores.

Problem: B=4, T=2048, D=1024, H=16 heads of 64. Sharding: core c handles
batch c//2 and head-group c%2 (8 heads = 512 cols of the concat dim). Each
core computes its partial output projection o_g @ Wo_g^T; the host sums the
two partials per batch and adds the bias.

Per-core kernel (bf16 matmuls, fp32 accumulation):
  QT[hd, t], KT[hd, t]; V[t, hd] with a ones column per head (V_aug).
  scoresT[s, q] = K Q^T per stripe of 128 keys (causal lower block-row),
  exp fused into the PSUM->SBUF activation on ScalarE giving PT[s, q]
  stored in fp8e4 (ones-column denominator uses the same quantized values,
  so numerator/denominator errors track); causal mask = multiply the
  diagonal 128-block by an upper-triangular 0/1 mask after exp.
  PV: out[q, 0:65|65:130] = P @ V_aug accumulated over s-tiles; col 64 of
  each half is the softmax denominator. Normalize with a per-partition
  reciprocal, DMA-transpose [128,128] into oT[hd, t].
  proj: partial[t, :] = oT^T @ WoT_g.

Schedule: fp8 PT halves its SBUF footprint, so FOUR pt buffers fit and
each pair's PV can be deferred into the next pair's region. That balances
the PE against the ~42us/pair ScalarE exp stream:
  A (pair 0 scores): m0 chains k-major-pipelined against the input DMA,
     v tiles 0-11, m1 chains.
  B (pair 1 scores): v tiles 12-15, pv0, m2 chains.
  C (pair 2 scores): m3 chains, pv1, pv2.
  D (pair 3 scores): pv3 + output projection lagged two stripes.
A dummy exp at t=0 hoists the ~2.7us ACT table load; oT transposes
round-robin over three DMA queues to avoid descriptor-flood stalls.
"""

import numpy as np
import ml_dtypes
from contextlib import ExitStack

import concourse.mybir as mybir
import concourse.tile as tile
from concourse import bacc
from concourse.bass_utils import run_bass_kernel_spmd

F32 = mybir.dt.float32
BF16 = mybir.dt.bfloat16
F8 = mybir.dt.float8e4

FP8_PT = True  # store exp'd scores in fp8e4 (enables pv deferral)

B, T, D, H = 4, 2048, 1024, 16
HD = 64          # head dim
HG = 8           # heads per core
GW = HG * HD     # 512, group width
NT = T // 128    # 16 t-tiles
NK = D // 128    # 8 d-tiles
N_CORES = 8

PT_DT = F8 if FP8_PT else BF16

# ragged PT stripe offsets: stripe j holds cols q=128j..2048
_PT_OFF = [2048 * j - 64 * j * (j - 1) for j in range(NT + 1)]
PT_LEN = _PT_OFF[NT]  # 17408


def _build():
    nc = bacc.Bacc("TRN2", target_bir_lowering=False, debug=False,
                   num_devices=N_CORES)
    xT_d = nc.dram_tensor("xT", [D, T], BF16, kind="ExternalInput").ap()
    wq_d = nc.dram_tensor("wq", [D, GW], BF16, kind="ExternalInput").ap()
    wk_d = nc.dram_tensor("wk", [D, GW], BF16, kind="ExternalInput").ap()
    wv_d = nc.dram_tensor("wv", [D, GW], BF16, kind="ExternalInput").ap()
    wo_d = nc.dram_tensor("woT", [GW, D], BF16, kind="ExternalInput").ap()
    tri_d = nc.dram_tensor("tri", [128, 128], PT_DT, kind="ExternalInput").ap()
    out_d = nc.dram_tensor("out", [T, D], BF16, kind="ExternalOutput").ap()

    with tile.TileContext(nc) as tc, ExitStack() as ctx:
        perm = ctx.enter_context(tc.tile_pool(name="perm", bufs=1))
        psA = ctx.enter_context(tc.tile_pool(name="psA", bufs=2, space="PSUM"))
        psB = ctx.enter_context(tc.tile_pool(name="psB", bufs=2, space="PSUM"))
        ps_o = ctx.enter_context(tc.tile_pool(name="ps_o", bufs=2, space="PSUM"))

        tri = perm.tile([128, 128], PT_DT, tag="tri")

        qT = perm.tile([128, 4, T], BF16, tag="qT")
        kT = perm.tile([128, 4, T], BF16, tag="kT")
        vsb = perm.tile([128, NT, HG * (HD + 1)], BF16, tag="vsb")
        wob = perm.tile([128, 4, D], BF16, tag="wob")
        oT = perm.tile([128, 4, T], BF16, tag="oT")

        # dummy exp at t=0 pulls the ACT table load off the critical path
        dummy = perm.tile([128, 8], F32, tag="dummy")
        nc.vector.memset(dummy[:, 0:4], 0.0)
        nc.scalar.activation(dummy[:, 4:8], dummy[:, 0:4],
                             mybir.ActivationFunctionType.Exp, scale=1.0)

        # ones columns for V_aug
        vcols = vsb.rearrange("p j (h c) -> p j h c", c=HD + 1)
        nc.vector.memset(vcols[:, :, :, HD:HD + 1], 1.0)

        def scores_chunk(pair, pts, j, qa, w):
            """one <=1024-wide chunk of scoresT for both heads of a stripe
            (the heads' K=64 matmuls sit on disjoint PE row groups)"""
            m = pair
            pss = [psA.tile([128, 1024], F32, tag="psA",
                            name=f"s{pair}_{hh}_{j}_{qa}") for hh in range(2)]
            a = 0
            while a < w:
                b = min(a + 512, w)
                for hh in range(2):
                    base = 64 * hh
                    nc.tensor.matmul(
                        pss[hh][:, a:b],
                        kT[base:base + 64, m, 128 * j:128 * (j + 1)],
                        qT[base:base + 64, m, qa + a:qa + b],
                        start=True, stop=True,
                        tile_position=(base, 0))
                a = b
            o0 = _PT_OFF[j] + (qa - 128 * j)
            for hh in range(2):
                nc.scalar.activation(
                    pts[hh][:, o0:o0 + w], pss[hh][:, :w],
                    mybir.ActivationFunctionType.Exp, scale=0.125)
            if qa == 128 * j:
                # first chunk holds the diagonal block: causal mask
                for hh in range(2):
                    nc.vector.tensor_mul(pts[hh][:, o0:o0 + 128],
                                         pts[hh][:, o0:o0 + 128], tri[:])

        def stripe_chunks(j):
            out = []
            qa = 128 * j
            first = True
            while qa < T:
                w = min(1024 - (qa % 1024) if first else 1024, T - qa)
                first = False
                out.append((qa, w))
                qa += w
            return out

        def scores_stripe(pair, pts, j):
            for qa, w in stripe_chunks(j):
                scores_chunk(pair, pts, j, qa, w)

        tq_rr = [0]

        def pv_i(pair, pts, i, smp):
            """PV + normalize + DMA-transpose into oT for one q-tile"""
            po = ps_o.tile([128, 2 * (HD + 1)], F32, tag="po")
            for hh in range(2):
                h = 2 * pair + hh
                pt = pts[hh]
                c0 = (HD + 1) * hh
                for j in range(i + 1):
                    nc.tensor.matmul(
                        po[:, c0:c0 + HD + 1],
                        pt[:, _PT_OFF[j] + 128 * (i - j):
                           _PT_OFF[j] + 128 * (i - j) + 128],
                        vsb[:, j, (HD + 1) * h:(HD + 1) * (h + 1)],
                        start=(j == 0), stop=(j == i))
            recip = smp.tile([128, 2], F32, tag="recip")
            pov = po.rearrange("p (h c) -> p h c", c=HD + 1)
            nc.vector.reciprocal(recip[:], pov[:, :, HD])
            onat = smp.tile([128, 128], BF16, tag="onat")
            for hh in range(2):
                c0 = (HD + 1) * hh
                nc.vector.tensor_scalar_mul(
                    onat[:, 64 * hh:64 * hh + 64],
                    po[:, c0:c0 + HD], recip[:, hh:hh + 1])
            tq = [nc.sync, nc.scalar][tq_rr[0] % 2]
            tq_rr[0] += 1
            tq.dma_start(oT[:, pair, 128 * i:128 * (i + 1)],
                         onat[:], transpose=True)

        with tc.tile_pool(name="ph1", bufs=1) as ph1, \
             tc.tile_pool(name="ptp", bufs=(4 if FP8_PT else 2)) as ptp, \
             tc.tile_pool(name="sm", bufs=16) as smp, \
             tc.tile_pool(name="outp", bufs=2) as outp:
            xT = ph1.tile([128, NK, T], BF16, tag="xT")
            wqb = ph1.tile([128, NK, GW], BF16, tag="wqb")
            wkb = ph1.tile([128, NK, GW], BF16, tag="wkb")
            wvb = ph1.tile([128, NK, GW], BF16, tag="wvb")

            # k-major interleaved loads so the m0 chains pipeline with DMA
            qs = [nc.sync, nc.scalar, nc.gpsimd]
            for k in range(NK):
                qs[0].dma_start(xT[:, k, :], xT_d[128 * k:128 * (k + 1), :])
                qs[1].dma_start(wqb[:, k, :], wq_d[128 * k:128 * (k + 1), :])
                qs[2].dma_start(wkb[:, k, :], wk_d[128 * k:128 * (k + 1), :])
            qs[1].dma_start(tri[:], tri_d[:])
            for k in range(NK):
                qs[(k % 2) + 1].dma_start(wvb[:, k, :],
                                          wv_d[128 * k:128 * (k + 1), :])
            for k in range(4):
                qs[0].dma_start(wob[:, k, :], wo_d[128 * k:128 * (k + 1), :])

            def qk_chain(mt, c, which):
                """one 8-matmul contraction chain: q (which=0) or k proj,
                m-tile mt, column chunk c (512 wide)"""
                wbt, dst = ((wqb, qT), (wkb, kT))[which]
                ps = psB.tile([128, 512], F32, tag="psB")
                for k in range(NK):
                    nc.tensor.matmul(
                        ps[:], wbt[:, k, 128 * mt:128 * (mt + 1)],
                        xT[:, k, 512 * c:512 * (c + 1)],
                        start=(k == 0), stop=(k == NK - 1))
                nc.vector.tensor_copy(dst[:, mt, 512 * c:512 * (c + 1)], ps[:])

            def v_jtile(j):
                ps = psB.tile([128, 512], F32, tag="psB")
                for k in range(NK):
                    nc.tensor.matmul(ps[:],
                                     xT[:, k, 128 * j:128 * (j + 1)],
                                     wvb[:, k, :],
                                     start=(k == 0), stop=(k == NK - 1))
                nc.vector.tensor_copy(vcols[:, j, :, :HD], ps[:])

            def proj_i(i):
                ost = outp.tile([128, D], BF16, tag="ost", name=f"ost{i}")
                for n in range(2):
                    ps = psB.tile([128, 512], F32, tag="psB")
                    for k in range(4):
                        nc.tensor.matmul(ps[:],
                                         oT[:, k, 128 * i:128 * (i + 1)],
                                         wob[:, k, 512 * n:512 * (n + 1)],
                                         start=(k == 0), stop=(k == 3))
                    nc.vector.tensor_copy(ost[:, 512 * n:512 * (n + 1)],
                                          ps[:])
                nc.scalar.dma_start(out_d[128 * i:128 * (i + 1), :], ost[:])

            def pt_alloc(pair):
                return [ptp.tile([128, PT_LEN], PT_DT, tag="pt",
                                 name=f"pt{pair}_{hh}") for hh in range(2)]

            # ---- region A: pair 0 scores ----
            pts0 = pt_alloc(0)
            # m0 chains for q/k cols 0:1024, k-major so they pipeline
            # against the arriving xT/wq/wk k-tiles
            psq01 = psA.tile([128, 1024], F32, tag="psA", name="pro_q01")
            psk01 = psA.tile([128, 1024], F32, tag="psA", name="pro_k01")
            for k in range(NK):
                st, sp = (k == 0), (k == NK - 1)
                for c in range(2):
                    nc.tensor.matmul(psq01[:, 512 * c:512 * (c + 1)],
                                     wqb[:, k, 0:128],
                                     xT[:, k, 512 * c:512 * (c + 1)],
                                     start=st, stop=sp)
                    nc.tensor.matmul(psk01[:, 512 * c:512 * (c + 1)],
                                     wkb[:, k, 0:128],
                                     xT[:, k, 512 * c:512 * (c + 1)],
                                     start=st, stop=sp)
            nc.vector.tensor_copy(qT[:, 0, 0:1024], psq01[:])
            nc.vector.tensor_copy(kT[:, 0, 0:1024], psk01[:])
            scores_chunk(0, pts0, 0, 0, 1024)
            for c in range(2, 4):
                qk_chain(0, c, 0)
                qk_chain(0, c, 1)
            scores_chunk(0, pts0, 0, 1024, 1024)
            v_jtile(0)
            if not FP8_PT:
                pv_i(0, pts0, 0, smp)
            for j in range(1, NT):
                scores_stripe(0, pts0, j)
                if j < 12:
                    v_jtile(j)
                if j >= 8:
                    qk_chain(1, (j - 8) // 2, (j - 8) % 2)
                if not FP8_PT:
                    pv_i(0, pts0, j, smp)

            # ---- region B: pair 1 scores (+ pv0 when deferred) ----
            pts1 = pt_alloc(1)
            for j in range(NT):
                scores_stripe(1, pts1, j)
                if j < 4:
                    v_jtile(12 + j)
                if j >= 8:
                    qk_chain(2, (j - 8) // 2, (j - 8) % 2)
                pv_i(0 if FP8_PT else 1, pts0 if FP8_PT else pts1, j, smp)

            # ---- region C: pair 2 scores + pv1/pv2 ----
            pts2 = pt_alloc(2)
            for j in range(NT):
                scores_stripe(2, pts2, j)
                if j < 8:
                    qk_chain(3, j // 2, j % 2)
                if FP8_PT:
                    pv_i(1, pts1, j, smp)
                pv_i(2, pts2, j, smp)

            # ---- region D: pair 3 scores + pv3 + lagged out proj ----
            pts3 = pt_alloc(3)
            for j in range(NT):
                scores_stripe(3, pts3, j)
                pv_i(3, pts3, j, smp)
                if j >= 2:
                    proj_i(j - 2)
            proj_i(NT - 2)
            proj_i(NT - 1)

    nc.compile()
    return nc


_NC_CACHE = None


def _get_nc():
    global _NC_CACHE
    if _NC_CACHE is None:
        _NC_CACHE = _build()
    return _NC_CACHE


def _prep_in_maps(x, Wq, Wk, Wv, Wo):
    bf = ml_dtypes.bfloat16
    tri_dt = ml_dtypes.float8_e4m3 if FP8_PT else bf
    tri = np.triu(np.ones((128, 128), dtype=tri_dt))
    in_maps = []
    for c in range(N_CORES):
        b, g = c // 2, c % 2
        hsl = slice(HG * g, HG * (g + 1))
        in_maps.append({
            "xT": np.ascontiguousarray(x[b].T).astype(bf),
            "wq": np.ascontiguousarray(
                Wq[hsl].transpose(1, 0, 2).reshape(D, GW)).astype(bf),
            "wk": np.ascontiguousarray(
                Wk[hsl].transpose(1, 0, 2).reshape(D, GW)).astype(bf),
            "wv": np.ascontiguousarray(
                Wv[hsl].transpose(1, 0, 2).reshape(D, GW)).astype(bf),
            "woT": np.ascontiguousarray(
                Wo[:, GW * g:GW * (g + 1)].T).astype(bf),
            "tri": tri,
        })
    return in_maps


def kernel(x, Wq, Wk, Wv, Wo, bo, _trace=False, _tmpdir=None):
    nc = _get_nc()
    x = np.asarray(x, dtype=np.float32)
    bo = np.asarray(bo, dtype=np.float32)
    in_maps = _prep_in_maps(x, np.asarray(Wq, np.float32),
                            np.asarray(Wk, np.float32),
                            np.asarray(Wv, np.float32),
                            np.asarray(Wo, np.float32))
    res = run_bass_kernel_spmd(nc, in_maps, core_ids=list(range(N_CORES)),
                               trace=_trace, tmpdir=_tmpdir)
    out = np.empty((B, T, D), dtype=np.float32)
    for b in range(B):
        out[b] = (res.results[2 * b]["out"].astype(np.float32)
                  + res.results[2 * b + 1]["out"].astype(np.float32) + bo)
    if _trace:
        return out, res
    return out
